# revision 11
# baseline (speedup 1.0000x reference)
"""CASVDDenseMul fused kernel for 8 Trainium2 NeuronCores.

Reference computation (fp32):
    chi = sigmoid(context @ W + B)          # [B, R]
    t   = (inputs @ U) * (S * chi)          # [B, R]
    out = relu(t @ V.T + 2*bias)            # [B, UNITS]

Sharding: data-parallel over batch; each of the 8 cores handles 512 rows.
All factor weights (U, S, V, W, B, bias) are replicated.

Layout choices (all host-side, free):
  - x and context are fed transposed ([feature, batch]) so the contraction
    dim lands on SBUF partitions with no on-device transposes.
  - V is fed transposed ([R, UNITS]) so it can act as the matmul stationary
    operand per m-tile.
  - The kernel computes out.T [UNITS, batch] per core (bias is then a
    per-partition scalar, fusing bias+relu into one scalar-engine op);
    the host transposes back.
  - All matmul operands are pre-rounded to the fp32r grid (11-bit
    mantissa) on the host; the PE consumes fp32r at twice the fp32 rate.
  - U and xT are packed into one DRAM tensor ("ux"), W and ctxT into
    another ("wctx"), so each contraction chunk arrives with a single
    DMA (the Sync engine's ~0.6us per-DMA issue cost otherwise paces
    the whole input stream below HBM rate).
"""

import numpy as np

from concourse import bacc, mybir
from concourse import tile
from concourse.bass_utils import run_bass_kernel_spmd

N_CORES = 8
B_SZ, N_IN, N_CTX, UNITS, RANK = 4096, 4096, 512, 4096, 256
BS = B_SZ // N_CORES  # 512 batch rows per core

P = 128
KC_IN = N_IN // P    # 32 contraction chunks for x @ U
KC_CTX = N_CTX // P  # 4  contraction chunks for ctx @ W
RT = RANK // P       # 2  rank tiles
MT = UNITS // P      # 32 unit (output) tiles
UXW = RANK + BS      # 768 packed columns per k-chunk
# k-chunk DMA groups: big groups early (cheap issue), single chunks at the
# tail so the PE's last mm1 steps aren't gated on a wide transfer.
UX_GROUPS = [[k, k + 1] for k in range(0, 28, 2)] + [[28], [29], [30], [31]]
VT_AFTER_GROUP = 10  # issue the VT DMAs after this many ux groups

FP32 = mybir.dt.float32
FP32R = mybir.dt.float32r


def _build_nc():
    nc = bacc.Bacc("TRN2", target_bir_lowering=False, debug=False, enable_asserts=False)

    ux = nc.declare_dram_parameter("ux", [KC_IN, P, UXW], FP32R, isOutput=False)
    wctx = nc.declare_dram_parameter("wctx", [P, KC_CTX, UXW], FP32R, isOutput=False)
    VT = nc.declare_dram_parameter("VT", [RANK, UNITS], FP32R, isOutput=False)
    consts = nc.declare_dram_parameter("consts", [P, 2 * RT + MT], FP32, isOutput=False)
    outT = nc.declare_dram_parameter("outT", [UNITS, BS], FP32, isOutput=True)

    out_tiles = outT.rearrange("(m p) b -> m p b", p=P)   # [32, 128, 512]

    with tile.TileContext(nc) as tc:
        with (
            tc.tile_pool(name="cpool", bufs=1) as cpool,
            tc.tile_pool(name="small", bufs=1) as small,
            tc.tile_pool(name="stream", bufs=1) as stream,
            tc.tile_pool(name="acts", bufs=1) as acts,
            tc.tile_pool(name="ostage", bufs=8) as ostage,
            tc.tile_pool(name="pchi", bufs=1, space="PSUM") as pchi,
            tc.tile_pool(name="pt", bufs=1, space="PSUM") as pt,
            tc.tile_pool(name="pout", bufs=5, space="PSUM") as pout,
        ):
            # ---- small weights + constants ----
            wctx_sb = small.tile([P, KC_CTX, UXW], FP32R, tag="wctx")
            for k in range(KC_CTX):
                nc.sync.dma_start(wctx_sb[:, k, :], wctx[:, k, :])
            c_sb = cpool.tile([P, 2 * RT + MT], FP32, tag="consts")
            nc.sync.dma_start(c_sb[:], consts[:])
            s2_sb = c_sb[:, 0:RT]
            b2_sb = c_sb[:, RT:2 * RT]
            bias_sb = c_sb[:, 2 * RT:]

            # chi.T = sigmoid(W.T @ ctxT + B) * S -- emitted interleaved
            # with the mm1 stream below so its matmuls fill the PE's
            # DMA-wait gaps instead of delaying mm1's start.
            s_chi = [acts.tile([P, BS], FP32, tag=f"schi{rt}", name=f"schi{rt}")
                     for rt in range(RT)]

            def emit_chi(rt):
                psum_chi = pchi.tile([P, BS], FP32, tag="chi", name="psum_chi")
                for k in range(KC_CTX):
                    nc.tensor.matmul(
                        psum_chi[:],
                        wctx_sb[:, k, rt * P:(rt + 1) * P],
                        wctx_sb[:, k, RANK:],
                        start=(k == 0),
                        stop=(k == KC_CTX - 1),
                        skip_group_check=True,
                    )
                nc.scalar.activation(
                    s_chi[rt][:], psum_chi[:],
                    mybir.ActivationFunctionType.Sigmoid,
                    bias=b2_sb[:, rt:rt + 1], scale=1.0,
                )
                nc.vector.tensor_scalar_mul(
                    s_chi[rt][:], s_chi[rt][:], s2_sb[:, rt:rt + 1]
                )

            # ---- t.T = (U.T @ xT) * s_chi   (stream packed u/x groups) ----
            ux_tiles = [stream.tile([P, len(ks), UXW], FP32R,
                                    tag=f"ux{g}", name=f"ux{g}")
                        for g, ks in enumerate(UX_GROUPS)]
            vt_sb = small.tile([P, RT, UNITS], FP32R, tag="vt")
            for g, ks in enumerate(UX_GROUPS):
                # alternate the two HWDGE rings so one ring's issue rate
                # doesn't cap the input stream
                eng = nc.sync if g % 2 == 0 else nc.scalar
                eng.dma_start(
                    ux_tiles[g][:],
                    ux[ks[0]:ks[0] + len(ks)].rearrange("k p w -> p k w"),
                )
                if g == VT_AFTER_GROUP:
                    # VT lands right before mm2 needs it, without delaying
                    # the u/x stream at the start.
                    for c in range(RT):
                        nc.scalar.dma_start(
                            vt_sb[:, c, :],
                            VT.rearrange("(c p) m -> c p m", p=P)[c],
                        )

            psum_t = [pt.tile([P, BS], FP32, tag=f"t{rt}", name=f"pt{rt}")
                      for rt in range(RT)]
            for g, ks in enumerate(UX_GROUPS):
                for j, k in enumerate(ks):
                    for rt in range(RT):
                        nc.tensor.matmul(
                            psum_t[rt][:],
                            ux_tiles[g][:, j, rt * P:(rt + 1) * P],
                            ux_tiles[g][:, j, RANK:],
                            start=(k == 0),
                            stop=(k == KC_IN - 1),
                            skip_group_check=True,
                        )
                if g == 1:
                    emit_chi(0)
                elif g == 3:
                    emit_chi(1)

            t_sb = [acts.tile([P, BS], FP32R, tag=f"t_sb{rt}", name=f"t_sb{rt}")
                    for rt in range(RT)]
            for rt in range(RT):
                nc.vector.tensor_mul(t_sb[rt][:], psum_t[rt][:], s_chi[rt][:])

            # ---- out.T = relu(V @ t.T + 2*bias) per 128-unit tile ----
            for m in range(MT):
                psum_o = pout.tile([P, BS], FP32, tag="o")
                for c in range(RT):
                    nc.tensor.matmul(
                        psum_o[:],
                        vt_sb[:, c, m * P:(m + 1) * P],
                        t_sb[c][:],
                        start=(c == 0),
                        stop=(c == RT - 1),
                    )
                o_sb = ostage.tile([P, BS], FP32, tag="osb")
                if m % 2 == 0:
                    nc.scalar.activation(
                        o_sb[:], psum_o[:],
                        mybir.ActivationFunctionType.Relu,
                        bias=bias_sb[:, m:m + 1], scale=1.0,
                    )
                else:
                    # split PSUM-evacuation load between ACT and DVE
                    nc.vector.tensor_scalar(
                        o_sb[:], psum_o[:],
                        bias_sb[:, m:m + 1], 0.0,
                        op0=mybir.AluOpType.add, op1=mybir.AluOpType.max,
                    )
                # alternate the two HWDGE rings so out-DMA issue keeps up
                eng = nc.sync if m % 2 == 0 else nc.scalar
                eng.dma_start(out_tiles[m], o_sb[:])

    nc.finalize()
    return nc


_NC_CACHE = {}


def _get_nc():
    if "nc" not in _NC_CACHE:
        _NC_CACHE["nc"] = _build_nc()
    return _NC_CACHE["nc"]


def _round_fp32r(a):
    """Round fp32 to the fp32r grid (11-bit mantissa; low 12 bits zero).

    The PE reads only the top 20 bits of each fp32r word; pre-rounding on
    the host gives round-to-nearest instead of hardware truncation.
    """
    u = np.ascontiguousarray(a, dtype=np.float32).view(np.uint32)
    r = (u + np.uint32(0x7FF) + ((u >> np.uint32(12)) & np.uint32(1))) & np.uint32(0xFFFFF000)
    return r.view(np.float32)


def _prepare_in_maps(inputs, context, U, S, V, W, B, bias):
    inputs = np.asarray(inputs, dtype=np.float32)
    context = np.asarray(context, dtype=np.float32)

    xTr = _round_fp32r(inputs.T)                 # [N_IN, B]
    ctxTr = _round_fp32r(context.T)              # [N_CTX, B]
    Ur = _round_fp32r(np.asarray(U, dtype=np.float32))
    Wr = _round_fp32r(np.asarray(W, dtype=np.float32))
    VTr = _round_fp32r(np.asarray(V, dtype=np.float32).T)  # [R, UNITS]

    S2 = np.asarray(S, dtype=np.float32).reshape(RT, P).T
    B2 = np.asarray(B, dtype=np.float32).reshape(RT, P).T
    bias2 = (2.0 * np.asarray(bias, dtype=np.float32)).reshape(MT, P).T
    consts = np.ascontiguousarray(
        np.concatenate([S2, B2, bias2], axis=1)
    )  # [128, 2+2+32]

    # U chunks, shared across cores: [KC_IN, P, RANK]
    u_g = Ur.reshape(KC_IN, P, RANK)
    # W chunks: [P, KC_CTX, RANK]
    w_g = Wr.reshape(KC_CTX, P, RANK).transpose(1, 0, 2)

    in_maps = []
    for c in range(N_CORES):
        sl = slice(c * BS, (c + 1) * BS)
        x_g = xTr[:, sl].reshape(KC_IN, P, BS)
        ux = np.empty((KC_IN, P, UXW), dtype=np.float32)
        ux[:, :, :RANK] = u_g
        ux[:, :, RANK:] = x_g
        ctx_g = ctxTr[:, sl].reshape(KC_CTX, P, BS).transpose(1, 0, 2)
        wctx = np.empty((P, KC_CTX, UXW), dtype=np.float32)
        wctx[:, :, :RANK] = w_g
        wctx[:, :, RANK:] = ctx_g
        in_maps.append({
            "ux": ux,
            "wctx": wctx,
            "VT": VTr,
            "consts": consts,
        })
    return in_maps


def _gather_out(results):
    out = np.empty((B_SZ, UNITS), dtype=np.float32)
    for c in range(N_CORES):
        out[c * BS:(c + 1) * BS, :] = results[c]["outT"].T
    return out


def kernel(inputs, context, U, S, V, W, B, bias):
    in_maps = _prepare_in_maps(inputs, context, U, S, V, W, B, bias)
    nc = _get_nc()
    res = run_bass_kernel_spmd(nc, in_maps, list(range(N_CORES)))
    return _gather_out(res.results)
